# revision 1
# baseline (speedup 1.0000x reference)
"""Trainium2 Bass kernel for a ViT/Swin-style transformer block.

Strategy: pure data-parallel over batch (64 -> 8 per core), no collectives.
On-device layout is feature-major ("transposed"): activations live as
[features(partitions, k-tiles of 128), tokens(free)].  All GEMMs run in bf16
with fp32 PSUM accumulation.  LayerNorm affine params and attention scale /
gamma factors are folded into the weights on the host; host also pre-gathers
the relative-position-bias table into dense per-head [m, n] maps.

SBUF is managed with one long-lived pool whose tag-slots are reused across
phases (Tile serializes the reuse via its dependency tracking):
  tg_x : xT fp32, residual accumulated in place        (phase A..end)
  tg_1 : kT (A..B)   -> h2   (C..D)
  tg_2 : qT (A..B)   -> gelu chunk buffer (D)
  tg_3 : v  (A..B)   -> w2 first half  (D)
  tg_4 : aoT (B..C)  -> w2 second half (D)
  tg_5 : h1 (A)      -> rpb (B) -> w1 first half (D)
Transient pools (weights, chunk temps) nest stack-wise per phase.
"""

import numpy as np
import ml_dtypes
from contextlib import ExitStack

import concourse.bacc as bacc
import concourse.bass as bass
import concourse.mybir as mybir
import concourse.tile as tile
from concourse.bass_utils import run_bass_kernel_spmd

bf16 = ml_dtypes.bfloat16
dt = mybir.dt
AF = mybir.ActivationFunctionType
ALU = mybir.AluOpType

# ---- problem dims (hardcoded) ----
B, N, D, H, DH, HID = 64, 197, 768, 12, 64, 3072
NCORES = 8
BPC = B // NCORES          # 8 batch elements per core
T = BPC * N                # 1576 token-columns per core
KT = D // 128              # 6 feature k-tiles
HT = HID // 128            # 24 hidden tiles
NCHUNK = 4
CHUNK = T // NCHUNK        # 394
MT = 2                     # m-tiles per batch element (128 + 69)
MSZ = [128, N - 128]       # [128, 69]
EPS = 1e-5

_NC_CACHE = {}


def _build_nc():
    if "nc" in _NC_CACHE:
        return _NC_CACHE["nc"]
    nc = bacc.Bacc(None, target_bir_lowering=False)

    # ---- DRAM I/O ----
    d_xT = nc.dram_tensor("xT", [D, T], dt.float32, kind="ExternalInput")
    d_wqkv = nc.dram_tensor("wqkvT", [D, 3 * D], dt.bfloat16, kind="ExternalInput")
    d_wp = nc.dram_tensor("wpT", [D, D], dt.bfloat16, kind="ExternalInput")
    d_w1 = nc.dram_tensor("w1T", [D, HID], dt.bfloat16, kind="ExternalInput")
    d_w2 = nc.dram_tensor("w2T", [HID, D], dt.bfloat16, kind="ExternalInput")
    d_qb = nc.dram_tensor("qb", [128, KT], dt.float32, kind="ExternalInput")
    d_kb = nc.dram_tensor("kb", [128, KT], dt.float32, kind="ExternalInput")
    d_vb = nc.dram_tensor("vb", [1, D], dt.bfloat16, kind="ExternalInput")
    d_pb = nc.dram_tensor("pb", [128, KT], dt.float32, kind="ExternalInput")
    d_b1 = nc.dram_tensor("b1", [128, HT], dt.float32, kind="ExternalInput")
    d_b2 = nc.dram_tensor("b2", [128, KT], dt.float32, kind="ExternalInput")
    d_rpb = nc.dram_tensor("rpbT", [128, H, MT * N], dt.bfloat16, kind="ExternalInput")
    d_yT = nc.dram_tensor("yT", [D, T], dt.float32, kind="ExternalOutput")

    with ExitStack() as ctx:
        tc = ctx.enter_context(tile.TileContext(nc))

        p_const = tc.alloc_tile_pool(name="const", bufs=1)
        p_rows = tc.alloc_tile_pool(name="prows", bufs=2)
        p_big = tc.alloc_tile_pool(name="pbig", bufs=1)

        # constants
        ones_mu = p_const.tile([128, 1], dt.bfloat16)      # 1/768 for mean sums
        ones_c = p_const.tile([128, 1], dt.bfloat16)       # 1.0 column
        ones_r = p_const.tile([1, 128], dt.bfloat16)       # 1.0 row
        eps_t = p_const.tile([1, 1], dt.float32)
        nc.vector.memset(ones_mu[:], 1.0 / D)
        nc.vector.memset(ones_c[:], 1.0)
        nc.vector.memset(ones_r[:], 1.0)
        nc.vector.memset(eps_t[:], EPS)
        t_qb = p_const.tile([128, KT], dt.float32)
        t_kb = p_const.tile([128, KT], dt.float32)
        t_vb = p_const.tile([1, D], dt.bfloat16)
        t_pb = p_const.tile([128, KT], dt.float32)
        t_b1 = p_const.tile([128, HT], dt.float32)
        t_b2 = p_const.tile([128, KT], dt.float32)
        for t_, d_ in [(t_qb, d_qb), (t_kb, d_kb), (t_vb, d_vb),
                       (t_pb, d_pb), (t_b1, d_b1), (t_b2, d_b2)]:
            nc.gpsimd.dma_start(t_[:], d_[:])

        # long-lived slots
        xT = p_big.tile([128, KT, T], dt.float32, tag="tg_x")
        xTr = d_xT.rearrange("(k p) t -> p k t", p=128)
        for k in range(KT):
            nc.gpsimd.dma_start(xT[:, k, :], xTr[:, k, :])
        kTt = p_big.tile([128, KT, T], dt.bfloat16, tag="tg_1")
        qT = p_big.tile([128, KT, T], dt.bfloat16, tag="tg_2")
        # v is token-major in per-head blocks of 65 columns (64 v + ones col)
        # so the softmax denominator rides along in the AV matmul as row 64.
        vtok = p_big.tile([128, BPC, MT, H, 65], dt.bfloat16, tag="tg_3")
        for h in range(H):
            nc.vector.memset(vtok[:, :, :, h, 64:65], 1.0)
        h1 = p_big.tile([128, KT, T], dt.bfloat16, tag="tg_5")
        rpb = p_big.tile([128, H, MT * N], dt.bfloat16, tag="tg_rpb")
        nc.gpsimd.dma_start(rpb[:], d_rpb[:])

        # ============ LayerNorm helper (feature-major) ============
        def layernorm(src_f32, dst_bf, tmp_pool, psum_pool):
            for c in range(NCHUNK):
                cs = bass.ts(c, CHUNK)
                xb = tmp_pool.tile([128, KT, CHUNK], dt.bfloat16, tag="xbf", bufs=1)
                x2 = tmp_pool.tile([128, KT, CHUNK], dt.bfloat16, tag="x2", bufs=1)
                for k in range(KT):
                    nc.scalar.copy(xb[:, k, :], src_f32[:, k, cs])
                    nc.scalar.square(x2[:, k, :], src_f32[:, k, cs])
                mu_ps = psum_pool.tile([1, CHUNK], dt.float32, tag="stat", bufs=2)
                ms_ps = psum_pool.tile([1, CHUNK], dt.float32, tag="stat", bufs=2)
                for k in range(KT):
                    nc.tensor.matmul(mu_ps[:], ones_mu[:], xb[:, k, :],
                                     start=(k == 0), stop=(k == KT - 1))
                for k in range(KT):
                    nc.tensor.matmul(ms_ps[:], ones_mu[:], x2[:, k, :],
                                     start=(k == 0), stop=(k == KT - 1))
                musq = p_rows.tile([1, CHUNK], dt.float32, tag="musq")
                nc.scalar.square(musq[:], mu_ps[:])
                var = p_rows.tile([1, CHUNK], dt.float32, tag="var")
                nc.vector.tensor_sub(var[:], ms_ps[:], musq[:])
                std = p_rows.tile([1, CHUNK], dt.float32, tag="std")
                nc.scalar.activation(std[:], var[:], AF.Sqrt, bias=eps_t[0:1, 0:1])
                a_f = p_rows.tile([1, CHUNK], dt.float32, tag="af")
                nc.vector.reciprocal_approx_fast(a_f[:], std[:])
                a_b = p_rows.tile([1, CHUNK], dt.bfloat16, tag="ab")
                nc.vector.tensor_copy(a_b[:], a_f[:])
                b_b = p_rows.tile([1, CHUNK], dt.bfloat16, tag="bb")
                with nc.allow_low_precision(reason="LN shift row as bf16 bc-matmul rhs"):
                    nc.vector.scalar_tensor_tensor(b_b[:], mu_ps[:], -1.0, a_f[:],
                                                   op0=ALU.mult, op1=ALU.mult)
                bc_a = psum_pool.tile([128, CHUNK], dt.float32, tag="bc", bufs=2)
                bc_b = psum_pool.tile([128, CHUNK], dt.float32, tag="bc", bufs=2)
                nc.tensor.matmul(bc_a[:], ones_r[:], a_b[:], start=True, stop=True)
                nc.tensor.matmul(bc_b[:], ones_r[:], b_b[:], start=True, stop=True)
                for k in range(KT):
                    tmp = tmp_pool.tile([128, CHUNK], dt.float32, tag="ntmp", bufs=2)
                    nc.vector.tensor_mul(tmp[:], src_f32[:, k, cs], bc_a[:])
                    nc.vector.tensor_add(dst_bf[:, k, cs], tmp[:], bc_b[:])

        # ============ Phase A: LN1 + QKV ============
        p_qkvw = tc.alloc_tile_pool(name="pqkvw", bufs=1)
        p_atmp = tc.alloc_tile_pool(name="patmp", bufs=1)
        psA = tc.alloc_tile_pool(name="psA", bufs=1, space="PSUM")

        wqkv = p_qkvw.tile([128, KT, 3 * D], dt.bfloat16)
        nc.gpsimd.dma_start(wqkv[:], d_wqkv.rearrange("(k p) m -> p k m", p=128))

        layernorm(xT, h1, p_atmp, psA)

        for c in range(NCHUNK):
            cs = bass.ts(c, CHUNK)
            for d_i in range(KT):
                pq = psA.tile([128, CHUNK], dt.float32, tag="mm", bufs=4)
                for k in range(KT):
                    nc.tensor.matmul(pq[:], wqkv[:, k, bass.ts(d_i, 128)],
                                     h1[:, k, cs], start=(k == 0), stop=(k == KT - 1))
                nc.scalar.activation(qT[:, d_i, cs], pq[:], AF.Identity,
                                     bias=t_qb[:, d_i:d_i + 1])
            for d_i in range(KT):
                pk = psA.tile([128, CHUNK], dt.float32, tag="mm", bufs=4)
                for k in range(KT):
                    nc.tensor.matmul(pk[:], wqkv[:, k, D + d_i * 128:D + d_i * 128 + 128],
                                     h1[:, k, cs], start=(k == 0), stop=(k == KT - 1))
                nc.scalar.activation(kTt[:, d_i, cs], pk[:], AF.Identity,
                                     bias=t_kb[:, d_i:d_i + 1])
        for b in range(BPC):
            for mt in range(MT):
                msz = MSZ[mt]
                n0 = b * N + mt * 128
                for half in range(2):
                    hs = bass.ts(half, 384)
                    pv = psA.tile([128, 384], dt.float32, tag="mm", bufs=4)
                    nc.tensor.matmul(pv[0:msz, :], ones_r[0:1, 0:msz], t_vb[:, hs],
                                     start=True, stop=False)
                    for k in range(KT):
                        nc.tensor.matmul(pv[0:msz, :], h1[:, k, n0:n0 + msz],
                                         wqkv[:, k, 2 * D + half * 384:2 * D + half * 384 + 384],
                                         start=False, stop=(k == KT - 1))
                    nc.scalar.copy(
                        vtok[0:msz, b, mt, half * 6:(half + 1) * 6, 0:64],
                        pv[0:msz, :].rearrange("p (h w) -> p h w", w=64))

        psA.release()
        p_atmp.release()
        p_qkvw.release()

        # ============ Phase B: attention (3-stage pipelined head pairs) =====
        aoT = p_big.tile([128, KT, T], dt.bfloat16, tag="tg_4")
        p_aw = tc.alloc_tile_pool(name="paw", bufs=2)
        psB = tc.alloc_tile_pool(name="psB", bufs=1, space="PSUM")
        ones_sq = p_const.tile([128, 128], dt.bfloat16)
        nc.vector.memset(ones_sq[:], 1.0)

        def vaug_ap(b, mt, h):
            """lhsT [msz, 65]: head block = 64 v columns + ones column.
            AV output rows 0..63 = head AV, row 64 = softmax denominator."""
            return vtok[0:MSZ[mt], b, mt, h, :]

        PAIRS = [(b, j) for b in range(BPC) for j in range(H // 2)]
        state = {}

        def stage0(p):                       # scores -> psum (PE)
            b, j = p
            ts_n = slice(b * N, (b + 1) * N)
            scs = {}
            for e in range(2):
                hp = e * 64
                sc = psB.tile([128, MT * N], dt.float32, tag="sc", bufs=4,
                              name=f"sc_{b}_{j}_{e}")
                for mt in range(MT):
                    msz = MSZ[mt]
                    m0 = b * N + mt * 128
                    nc.tensor.matmul(sc[0:msz, bass.ts(mt, N)],
                                     kTt[hp:hp + 64, j, m0:m0 + msz],
                                     qT[hp:hp + 64, j, ts_n], start=True, stop=True)
                scs[e] = sc
            state[p] = {"scs": scs}

        def stage1(p):                       # +rpb, exp -> p_bf (DVE/ACT)
            b, j = p
            st = state[p]
            p_bf = p_aw.tile([128, 2, MT * N], dt.bfloat16, tag="pbf",
                             name=f"pbf_{b}_{j}")
            for e in range(2):
                sc = st["scs"][e]
                nc.vector.tensor_add(sc[:, :], sc[:, :], rpb[:, 2 * j + e, :])
                nc.scalar.activation(p_bf[:, e, :], sc[:, :], AF.Exp)
            st["p_bf"] = p_bf

        def stage2(p):                       # AV + denom (PE)
            b, j = p
            st = state[p]
            av = psB.tile([128, MT * N], dt.float32, tag="av", bufs=2,
                          name=f"av_{b}_{j}")
            for e in range(2):
                h = 2 * j + e
                for mt in range(MT):
                    nc.tensor.matmul(av[0:65, e * N:e * N + N], vaug_ap(b, mt, h),
                                     st["p_bf"][0:MSZ[mt], e, mt * N:mt * N + N],
                                     start=(mt == 0), stop=(mt == MT - 1))
            st["av"] = av

        def stage3(p):                       # recip, bcast, normalize (DVE/PE)
            b, j = p
            ts_n = slice(b * N, (b + 1) * N)
            st = state[p]
            # denominator rows (psum row 64, col-half e) -> base-0 sbuf rows;
            # reciprocal_approx_fast only addresses base partition 0 correctly.
            av = st["av"]
            bc = psB.tile([64, MT * N], dt.float32, tag="bcx", bufs=2,
                          name=f"bc_{b}_{j}")
            rr = p_aw.tile([1, MT * N], dt.float32, tag="rr", name=f"rr_{b}_{j}")
            nc.vector.tensor_copy(rr[:], av[64:65, :])
            rc = p_aw.tile([1, MT * N], dt.float32, tag="rc", name=f"rc_{b}_{j}")
            nc.vector.reciprocal_approx_fast(rc[:], rr[:])
            rcb = p_aw.tile([1, MT * N], dt.bfloat16, tag="rcb", name=f"rcb_{b}_{j}")
            with nc.allow_low_precision(reason="softmax 1/denom bf16 bc rhs"):
                nc.vector.tensor_copy(rcb[:], rc[:])
            for e in range(2):
                nc.tensor.matmul(bc[:, e * N:e * N + N], ones_sq[0:1, 0:64],
                                 rcb[0:1, e * N:e * N + N], start=True, stop=True)
            bcs = p_aw.tile([64, MT * N], dt.float32, tag="bcs", name=f"bcs_{b}_{j}")
            nc.scalar.copy(bcs[:], bc[:])
            for e in range(2):
                hp = e * 64
                nc.vector.tensor_mul(aoT[hp:hp + 64, j, ts_n],
                                     av[0:64, e * N:e * N + N],
                                     bcs[0:64, e * N:e * N + N])
            del state[p]

        NP = len(PAIRS)
        for i in range(NP + 2):
            if i - 2 >= 0:
                stage3(PAIRS[i - 2])
            if i - 1 >= 0 and i - 1 < NP:
                stage2(PAIRS[i - 1])
            if i < NP:
                stage0(PAIRS[i])
                stage1(PAIRS[i])

        p_aw.release()
        psB.release()

        # ============ Phase C: proj + residual1 (in place into xT) + LN2 ====
        h2 = p_big.tile([128, KT, T], dt.bfloat16, tag="tg_1")
        p_cw = tc.alloc_tile_pool(name="pcw", bufs=1)
        wp = p_cw.tile([128, KT, D], dt.bfloat16)
        nc.gpsimd.dma_start(wp[:], d_wp.rearrange("(k p) m -> p k m", p=128))
        p_ctmp = tc.alloc_tile_pool(name="pctmp", bufs=1)
        psC = tc.alloc_tile_pool(name="psC", bufs=1, space="PSUM")

        for c in range(NCHUNK):
            cs = bass.ts(c, CHUNK)
            for d_i in range(KT):
                pp = psC.tile([128, CHUNK], dt.float32, tag="mm", bufs=4)
                for k in range(KT):
                    nc.tensor.matmul(pp[:], wp[:, k, bass.ts(d_i, 128)],
                                     aoT[:, k, cs], start=(k == 0), stop=(k == KT - 1))
                nc.vector.scalar_tensor_tensor(xT[:, d_i, cs], pp[:],
                                               t_pb[:, d_i:d_i + 1], xT[:, d_i, cs],
                                               op0=ALU.add, op1=ALU.add)
        layernorm(xT, h2, p_ctmp, psC)

        psC.release()
        p_ctmp.release()
        p_cw.release()

        # ============ Phase D: MLP (res2 in place semantics via y tiles) ====
        w1a = p_big.tile([128, KT // 2, HID], dt.bfloat16, tag="tg_5")
        w2a = p_big.tile([128, HT // 2, D], dt.bfloat16, tag="tg_3")
        w2b = p_big.tile([128, HT // 2, D], dt.bfloat16, tag="tg_4")
        p_dw = tc.alloc_tile_pool(name="pdw", bufs=1)
        w1b = p_dw.tile([128, KT - KT // 2, HID], dt.bfloat16)
        p_y = tc.alloc_tile_pool(name="py", bufs=2)
        psD = tc.alloc_tile_pool(name="psD", bufs=1, space="PSUM")

        w1r = d_w1.rearrange("(k p) m -> p k m", p=128)
        nc.gpsimd.dma_start(w1a[:], w1r[:, 0:KT // 2, :])
        nc.gpsimd.dma_start(w1b[:], w1r[:, KT // 2:KT, :])
        w2r = d_w2.rearrange("(k p) m -> p k m", p=128)
        nc.gpsimd.dma_start(w2a[:], w2r[:, 0:HT // 2, :])
        nc.gpsimd.dma_start(w2b[:], w2r[:, HT // 2:HT, :])

        def w1_at(k):
            return (w1a[:, k, :] if k < KT // 2 else w1b[:, k - KT // 2, :])

        def w2_at(k):
            return (w2a[:, k, :] if k < HT // 2 else w2b[:, k - HT // 2, :])

        for c in range(NCHUNK):
            cs = bass.ts(c, CHUNK)
            g = p_big.tile([128, HT, CHUNK], dt.bfloat16, tag="tg_2")
            for hh in range(HT):
                pf = psD.tile([128, CHUNK], dt.float32, tag="f1", bufs=4)
                for k in range(KT):
                    nc.tensor.matmul(pf[:], w1_at(k)[:, bass.ts(hh, 128)],
                                     h2[:, k, cs], start=(k == 0), stop=(k == KT - 1))
                nc.scalar.activation(g[:, hh, :], pf[:], AF.Gelu,
                                     bias=t_b1[:, hh:hh + 1])
            y = p_y.tile([128, KT, CHUNK], dt.float32, tag="y")
            for d_i in range(KT):
                po = psD.tile([128, CHUNK], dt.float32, tag="f2", bufs=4)
                for k in range(HT):
                    nc.tensor.matmul(po[:], w2_at(k)[:, bass.ts(d_i, 128)],
                                     g[:, k, :], start=(k == 0), stop=(k == HT - 1))
                nc.vector.scalar_tensor_tensor(y[:, d_i, :], po[:],
                                               t_b2[:, d_i:d_i + 1], xT[:, d_i, cs],
                                               op0=ALU.add, op1=ALU.add)
            nc.gpsimd.dma_start(
                d_yT.rearrange("(k p) t -> p k t", p=128)[:, :, cs], y[:])

        psD.release()
        p_y.release()
        p_dw.release()
        p_big.release()
        p_rows.release()
        p_const.release()

    nc.finalize()
    _NC_CACHE["nc"] = nc
    return nc


def _prep_host(inputs):
    """Fold LN affines / scales / gammas into weights; build per-core in_maps."""
    f = np.float32
    x = np.asarray(inputs["x"], f)
    n1w, n1b = np.asarray(inputs["norm1_w"], f), np.asarray(inputs["norm1_b"], f)
    n2w, n2b = np.asarray(inputs["norm2_w"], f), np.asarray(inputs["norm2_b"], f)
    qkv_w = np.asarray(inputs["qkv_w"], f)
    q_bias, v_bias = np.asarray(inputs["q_bias"], f), np.asarray(inputs["v_bias"], f)
    rpb_table = np.asarray(inputs["rpb_table"], f)
    rel_index = np.asarray(inputs["rel_index"])
    proj_w, proj_b = np.asarray(inputs["proj_w"], f), np.asarray(inputs["proj_b"], f)
    g1, g2 = np.asarray(inputs["gamma1"], f), np.asarray(inputs["gamma2"], f)
    fc1_w, fc1_b = np.asarray(inputs["fc1_w"], f), np.asarray(inputs["fc1_b"], f)
    fc2_w, fc2_b = np.asarray(inputs["fc2_w"], f), np.asarray(inputs["fc2_b"], f)

    scale = DH ** -0.5
    Wq, Wk, Wv = qkv_w[0:D], qkv_w[D:2 * D], qkv_w[2 * D:3 * D]
    WqT = (scale * (Wq * n1w[None, :]).T).astype(bf16)
    WkT = ((Wk * n1w[None, :]).T).astype(bf16)
    WvT = ((Wv * n1w[None, :]).T).astype(bf16)
    wqkvT = np.ascontiguousarray(np.concatenate([WqT, WkT, WvT], axis=1))
    qb = (scale * (Wq @ n1b + q_bias)).reshape(KT, 128).T.copy()   # [128, KT]
    kb = (Wk @ n1b).reshape(KT, 128).T.copy()
    vb = (Wv @ n1b + v_bias).reshape(1, D).astype(bf16)
    wpT = np.ascontiguousarray((g1[:, None] * proj_w).T.astype(bf16))
    pb = (g1 * proj_b).reshape(KT, 128).T.copy()
    w1T = np.ascontiguousarray((fc1_w * n2w[None, :]).T.astype(bf16))
    b1 = (fc1_w @ n2b + fc1_b).reshape(HT, 128).T.copy()
    w2T = np.ascontiguousarray((g2[:, None] * fc2_w).T.astype(bf16))
    b2 = (g2 * fc2_b).reshape(KT, 128).T.copy()

    # rpbT[p, h, mt*N+n] = rpb[h, n, m=mt*128+p]  (scoresT orientation)
    RPB = rpb_table[rel_index]            # [n, m, H]
    rpbT = np.zeros((128, H, MT * N), f)
    for mt in range(MT):
        msz = MSZ[mt]
        blk = RPB[:, mt * 128:mt * 128 + msz, :].transpose(1, 2, 0)  # [m_sl, H, n]
        for h in range(H):
            rpbT[0:msz, h, mt * N:mt * N + N] = blk[:, h, :]
    rpbT = rpbT.astype(bf16)

    shared = dict(wqkvT=wqkvT, wpT=wpT, w1T=w1T, w2T=w2T,
                  qb=np.ascontiguousarray(qb), kb=np.ascontiguousarray(kb),
                  vb=vb, pb=np.ascontiguousarray(pb),
                  b1=np.ascontiguousarray(b1), b2=np.ascontiguousarray(b2),
                  rpbT=rpbT)
    in_maps = []
    for core in range(NCORES):
        xs = x[core * BPC:(core + 1) * BPC]            # [BPC, N, D]
        xT = np.ascontiguousarray(xs.reshape(T, D).T)  # [D, T]
        m = dict(shared)
        m["xT"] = xT
        in_maps.append(m)
    return in_maps


def kernel(**inputs) -> np.ndarray:
    nc = _build_nc()
    in_maps = _prep_host(inputs)
    res = run_bass_kernel_spmd(nc, in_maps, core_ids=list(range(NCORES)))
    outs = []
    for core in range(NCORES):
        yT = res.results[core]["yT"]                   # [D, T]
        outs.append(np.asarray(yT, np.float32).T.reshape(BPC, N, D))
    return np.concatenate(outs, axis=0)

